# revision 1
# baseline (speedup 1.0000x reference)
"""Trainium2 Bass kernel for CrossModalAttention.

Reference computation (per batch row b, modalities q,k in {0,1,2}):
  qp[m] = x[m] @ Wq[m] + bq[m];  kp[m] = x[m] @ Wk[m] + bk[m]
  scores[q,k] = v[q] . tanh(qp[q] + kp[k])          (k != q)
  alpha = softmax over k (2 off-diagonal entries per q)
  att[q] = sum_k alpha[q,k] * (x[k] @ Wt[q,k] + bt[q,k])
  fused  = LayerNorm(concat_m(x[m] + att[m]); gamma, beta)

Strategy: pure data parallel over the batch across 8 NeuronCores
(8192 rows per core), parameters replicated.  Per core, batch tiles of
128 rows; all matmuls in bf16 (fp32 PSUM accumulation), with x
transposed on-chip via PE-mode transposes so the embedding dim lands
on the partition (contraction) axis.  Biases are folded into the PSUM
accumulation as K=1 rank-1 matmuls.  The softmax over 2 entries is a
sigmoid of the score difference.

Changes vs the original baseline:
  - rstd via DVE-only Newton rsqrt (no ACT Sqrt): the Sqrt activation
    table lives in a different table set than Tanh/Sigmoid, so using it
    forced two ~2.7us ACT table reloads per 128-row tile.
  - qk projections in fp8-e4m3 with DoubleRow perf mode (2 contraction
    chunks per PE instruction).  The fp8 error only reaches the output
    through the softmax weights alpha, so it costs ~7e-3 rel (measured
    9.7e-3 total vs 2e-2 tolerance).  Wt matmuls stay bf16: fp8 there
    flows straight into the output and measured 2.3e-2 (fails).
  - tanh-input adds (qp[q]+kp[k]) moved to the otherwise-idle gpsimd
    engine (Pool); the bf16->fp8 cast of the transposed activations too.
  - attended kept in bf16; LN output written bf16 with an f32-upcast
    DMA store (total precision cost ~1e-3 rel).
  - software-pipelined emission: tile t's Wt/att/LN phase is emitted
    after tile t+1's load/transpose/qk/score phase, so the PE stream
    never stalls on the same tile's score chain or y-PSUM rotation.
  - bt2 bias folded into the residual (a1*bt1+a2*bt2 = a1*(bt1-bt2)+bt2
    since a2=1-a1): one DVE 2x bf16 add replaces 3 rank-1 PE matmuls.
  - PSUM pools retuned (qkp 2 / y 3 / transpose 2 banks).
Cost-model sim (8-tile shard): 133.6us -> 105.4us; engines balanced
(Act 63%, PE 61%, DVE 57%, Pool 46% busy).
"""

import json

import numpy as np

import concourse.bass as bass
import concourse.bass2jax as bass2jax
import concourse.bass_utils as bass_utils
import concourse.mybir as mybir
import concourse.tile as tile
from concourse.bass_utils import run_bass_kernel_spmd

M, E, A = 3, 512, 256
B_FULL = 65536
N_CORES = 8
BC = B_FULL // N_CORES  # 8192 rows per core
P = 128
EC = E // P  # 4 contraction chunks
LN_EPS = 1e-5

F32 = mybir.dt.float32
BF16 = mybir.dt.bfloat16
FP8 = mybir.dt.float8e4
AL = mybir.AluOpType
AF = mybir.ActivationFunctionType

# --- tuning flags -----------------------------------------------------------
QK_FP8 = True       # qk projections via fp8 DoubleRow matmuls
OUT_BF16 = True     # LN output in bf16, f32-upcast on the store DMA
SQ_ON_POOL = False  # LN square+reduce on gpsimd (Pool) instead of Act
OUT_ON_ACT = False  # LN normalize on Act (Identity w/ scale+bias) vs DVE
TIN_ON_POOL = True  # evacuate qkp on Act, tanh-input adds on gpsimd (Pool)

# For query modality q the two keys, in a fixed order.
K_FIRST = [1, 0, 0]
K_SECOND = [2, 2, 1]

# ---------------------------------------------------------------------------
# The walrus build in this container rejects instructions carrying more than
# one semaphore wait (limit varies by ISA struct; 1 is universally safe).
# Tile's wait-assignment freely emits several.  Legalize the serialized BIR:
# move excess waits onto NoOp instructions inserted just before the offender
# on the same engine — semantically identical (engine streams are in-order).
# ---------------------------------------------------------------------------
_MAX_WAITS = 1
_REAL_ENGINES = {"PE", "DVE", "Activation", "Pool", "SP"}


def _legalize_waits(bir_json) -> bytes:
    d = json.loads(bir_json)
    n_split = 0
    for f in d.get("functions", []):
        for b in f.get("blocks", []):
            insts = b.get("instructions", [])
            out = []
            for inst in insts:
                si = inst.get("sync_info")
                waits = (si or {}).get("on_wait") or []
                if len(waits) > _MAX_WAITS and inst.get("engine") in _REAL_ENGINES:
                    extra = waits[: len(waits) - _MAX_WAITS]
                    si["on_wait"] = waits[len(waits) - _MAX_WAITS :]
                    for j, w in enumerate(extra):
                        n_split += 1
                        out.append(
                            {
                                "debug": inst.get("debug", 0),
                                "engine": inst["engine"],
                                "ins": [],
                                "name": f"{inst['name']}-ws{j}",
                                "opcode": "NoOp",
                                "outs": [],
                                "sync_info": {"on_update": [], "on_wait": [w]},
                            }
                        )
                out.append(inst)
            b["instructions"] = out
    return json.dumps(d).encode()


_orig_compile_bir_kernel = bass_utils.compile_bir_kernel


def _patched_compile_bir_kernel(bir_json, tmpdir, neff_name="file.neff"):
    return _orig_compile_bir_kernel(_legalize_waits(bir_json), tmpdir, neff_name)


if bass_utils.compile_bir_kernel is not _patched_compile_bir_kernel:
    bass_utils.compile_bir_kernel = _patched_compile_bir_kernel
    bass2jax.compile_bir_kernel = _patched_compile_bir_kernel


def _build(bc: int, fast_gb: bool, reps: int = 1) -> bass.Bass:
    """Build the per-core program for a batch shard of `bc` rows.

    `reps` re-runs the whole batch loop that many times via a hardware
    loop — used only for benchmarking (amortizes dispatch overhead).
    """
    nt = bc // P
    nc = bass.Bass()

    x_d = nc.dram_tensor("x", [M, bc, E], F32, kind="ExternalInput")
    g_d = nc.dram_tensor("gamma", [M * E], F32, kind="ExternalInput")
    be_d = nc.dram_tensor("beta", [M * E], F32, kind="ExternalInput")
    # pre-arranged bf16 parameters (prepared host-side in kernel()):
    #   Wqk_bf[m, c] = [Wq[m][c-chunk] | Wk[m][c-chunk]]      [M, EC, P, 2A]
    #   Wt_bf[j, c]  = Wt[q_j, k_j][c-chunk]                  [6, EC, P, E]
    #   bqk_bf[m]    = [bq[m] | bk[m]]                        [M, 2A]
    #   btr_bf[j]    = bt[q_j, k_j]                           [6, E]
    #   v_bf[m]      = v[m]                                   [M, A]
    wqk_d = nc.dram_tensor("Wqk_bf", [M, EC, P, 2 * A], BF16, kind="ExternalInput")
    wtb_d = nc.dram_tensor("Wt_bf", [2 * M, EC, P, E], BF16, kind="ExternalInput")
    bqk_d = nc.dram_tensor("bqk_bf", [M, 2 * A], BF16, kind="ExternalInput")
    btr_d = nc.dram_tensor("btr_bf", [2 * M, E], BF16, kind="ExternalInput")
    vb_d = nc.dram_tensor("v_bf", [M, A], BF16, kind="ExternalInput")
    eye_d = nc.dram_tensor("eye_bf", [P, P], BF16, kind="ExternalInput")
    if QK_FP8:
        wqk8_d = nc.dram_tensor("Wqk_f8", [M, EC, P, 2 * A], FP8, kind="ExternalInput")
    out_d = nc.dram_tensor("out", [bc, M * E], F32, kind="ExternalOutput")

    pairs = [(q, K_FIRST[q]) for q in range(M)] + [(q, K_SECOND[q]) for q in range(M)]

    with tile.TileContext(nc) as tc:
        with (
            tc.tile_pool(name="const", bufs=1) as cpool,
            tc.tile_pool(name="xb", bufs=4) as xbpool,
            tc.tile_pool(name="xt", bufs=4) as xtpool,
            tc.tile_pool(name="tin", bufs=4) as tinpool,
            tc.tile_pool(name="tt", bufs=4) as ttpool,
            tc.tile_pool(name="tsc", bufs=4) as tscpool,
            tc.tile_pool(name="small", bufs=4) as smpool,
            tc.tile_pool(name="tmp", bufs=4) as tmppool,
            tc.tile_pool(name="att", bufs=4) as attpool,
            tc.tile_pool(name="sq", bufs=4) as sqpool,
            tc.tile_pool(name="outp", bufs=4) as outpool,
            tc.tile_pool(name="qkp_ps", bufs=2, space=bass.MemorySpace.PSUM) as qkpsum,
            tc.tile_pool(name="y_ps", bufs=3, space=bass.MemorySpace.PSUM) as ypsum,
            tc.tile_pool(name="tr_ps", bufs=2, space=bass.MemorySpace.PSUM) as trpsum,
        ):
            # ---- resident parameters (bf16, pre-arranged on host) ----
            # wqk[m]: per e-chunk c, cols [c*512, c*512+512) = [Wq[m] | Wk[m]] chunk.
            wqk = [
                cpool.tile([P, EC * 2 * A], BF16, name=f"wqk{m}", tag=f"wqk{m}")
                for m in range(M)
            ]
            for m in range(M):
                nc.sync.dma_start(
                    wqk[m][:, :].rearrange("p (c a) -> p c a", c=EC),
                    wqk_d[m].rearrange("c p a -> p c a"),
                )
            if QK_FP8:
                wqk8 = [
                    cpool.tile([P, EC * 2 * A], FP8, name=f"wqk8{m}", tag=f"wqk8{m}")
                    for m in range(M)
                ]
                for m in range(M):
                    nc.sync.dma_start(
                        wqk8[m][:, :].rearrange("p (c a) -> p c a", c=EC),
                        wqk8_d[m].rearrange("c p a -> p c a"),
                    )
            # wt[(q,k)]: per e-chunk c, cols [c*512,(c+1)*512) = Wt[q,k] chunk.
            wt = {}
            for j, (q, k) in enumerate(pairs):
                t = cpool.tile([P, EC * E], BF16, name=f"wt{q}_{k}", tag=f"wt{q}_{k}")
                wt[(q, k)] = t
                nc.sync.dma_start(
                    t[:, :].rearrange("p (c e) -> p c e", c=EC),
                    wtb_d[j].rearrange("c p e -> p c e"),
                )
            # bias rows
            bqk_row = [
                cpool.tile([1, 2 * A], BF16, name=f"bqk{m}", tag=f"bqk{m}")
                for m in range(M)
            ]
            for m in range(M):
                nc.sync.dma_start(
                    bqk_row[m][:1, :], bqk_d[m, :].rearrange("(o a) -> o a", o=1)
                )
            # bt bias, restructured (host-side) as:
            #   row j<M:  dbt[q]  = bt[q,k1] - bt[q,k2]   (rank-1 into y1)
            #   row j>=M: bt2[q]  = bt[q,k2]              (folded into x)
            # since a1*bt1 + a2*bt2 == a1*(bt1-bt2) + bt2  (a2 = 1-a1).
            bt_row = {}
            for j, (q, k) in enumerate(pairs):
                t = cpool.tile([1, E], BF16, name=f"bt{q}_{k}", tag=f"bt{q}_{k}")
                bt_row[(q, k)] = t
                nc.sync.dma_start(
                    t[:1, :], btr_d[j, :].rearrange("(o e) -> o e", o=1)
                )
            # ones rows for K=1 rank-1 matmuls (bias add, partition broadcast)
            ones_col = cpool.tile([1, P], BF16, name="ones", tag="ones")
            nc.vector.memset(ones_col[:1, :], 1.0)
            ones_f32 = cpool.tile([1, P], F32, name="onesf", tag="onesf")
            nc.vector.memset(ones_f32[:1, :], 1.0)
            # identity for PE-mode transposes
            eye = cpool.tile([P, P], BF16, name="eye", tag="eye")
            nc.sync.dma_start(eye[:, :], eye_d[:, :])

            # bt2 replicated across partitions via rank-1 matmuls: one
            # [P, M*E] tile so the fold into x is a single DVE add per tile
            bt2_rep = cpool.tile([P, M * E], BF16, name="bt2rep", tag="bt2rep")
            for q in range(M):
                ps = ypsum.tile([P, E], F32, name=f"b2c{q}", tag="y")
                nc.tensor.matmul(
                    ps[:, :], ones_col[:1, :], bt_row[(q, K_SECOND[q])][:1, :]
                )
                nc.scalar.copy(bt2_rep[:, q * E : (q + 1) * E], ps[:, :])

            # v replicated across partitions via rank-1 matmul
            v_rep = []
            for q in range(M):
                row = cpool.tile([1, A], BF16, name=f"vrow{q}", tag=f"vrow{q}")
                nc.sync.dma_start(
                    row[:1, :], vb_d[q, :].rearrange("(o a) -> o a", o=1)
                )
                ps = ypsum.tile([P, E], F32, name=f"vbc{q}", tag="y")
                nc.tensor.matmul(ps[:, 0:A], ones_col[:1, :], row[:1, :])
                rep = cpool.tile([P, A], BF16, name=f"vrep{q}", tag=f"vrep{q}")
                nc.scalar.copy(rep[:, :], ps[:, 0:A])
                v_rep.append(rep)

            # gamma/beta replicated (general path only)
            if not fast_gb:
                grow = cpool.tile([1, M * E], F32, name="grow", tag="grow")
                nc.gpsimd.dma_start(grow[:1, :], g_d[:].rearrange("(o e) -> o e", o=1))
                brow = cpool.tile([1, M * E], F32, name="brow", tag="brow")
                nc.gpsimd.dma_start(brow[:1, :], be_d[:].rearrange("(o e) -> o e", o=1))
                g_rep = cpool.tile([P, M * E], F32, name="grep", tag="grep")
                b_rep = cpool.tile([P, M * E], F32, name="brep", tag="brep")
                for src, dst in ((grow, g_rep), (brow, b_rep)):
                    for c in range(M):
                        cs = slice(c * E, (c + 1) * E)
                        ps = ypsum.tile([P, E], F32, name=f"gbc{c}", tag="y")
                        nc.tensor.matmul(ps[:, :], ones_f32[:1, :], src[:1, cs])
                        nc.vector.tensor_copy(dst[:, cs], ps[:, :])

            # ---- main loop over batch tiles ----
            import contextlib

            rep_ctx = (
                tc.For_i(0, reps, 1) if reps > 1 else contextlib.nullcontext()
            )
            with rep_ctx:
                # Software-pipelined emission: tile t's matmul-heavy second
                # phase (Wt matmuls, att combine, LN, store) is emitted after
                # tile t+1's first phase (load, transposes, qk projections,
                # score path), so the PE never sits idle waiting for the
                # score chain or the y-PSUM pool of the same tile.
                prev = None
                for ti in range(nt):
                    cur = _emit_phase1(
                        nc, tc, ti, fast_gb, pairs,
                        x_d, out_d, wqk, wqk8 if QK_FP8 else None, wt,
                        bqk_row, bt_row, bt2_rep, v_rep, ones_col, eye,
                        g_rep if not fast_gb else None,
                        b_rep if not fast_gb else None,
                        xbpool, xtpool, tinpool, ttpool, tscpool,
                        smpool, tmppool, attpool, sqpool, outpool,
                        qkpsum, ypsum, trpsum,
                    )
                    if prev is not None:
                        _emit_phase2(
                            nc, tc, prev, fast_gb, pairs,
                            out_d, wt, bt_row, ones_col,
                            g_rep if not fast_gb else None,
                            b_rep if not fast_gb else None,
                            smpool, tmppool, attpool, sqpool, outpool, ypsum,
                        )
                    prev = cur
                _emit_phase2(
                    nc, tc, prev, fast_gb, pairs,
                    out_d, wt, bt_row, ones_col,
                    g_rep if not fast_gb else None,
                    b_rep if not fast_gb else None,
                    smpool, tmppool, attpool, sqpool, outpool, ypsum,
                )

    return nc


def _emit_phase1(
    nc, tc, ti, fast_gb, pairs,
    x_d, out_d, wqk, wqk8, wt, bqk_row, bt_row, bt2_rep, v_rep, ones_col, eye,
    g_rep, b_rep,
    xbpool, xtpool, tinpool, ttpool, tscpool,
    smpool, tmppool, attpool, sqpool, outpool, qkpsum, ypsum, trpsum,
):
    bs = slice(ti * P, (ti + 1) * P)

    # load x for all modalities in one cast DMA (f32 -> bf16):
    # xb_all[p, m*E + e] = x[m, bs.start+p, e]
    xb_all = xbpool.tile([P, M * E], BF16, name="xball", tag="xball")
    nc.gpsimd.dma_start(
        xb_all[:, :].rearrange("p (m e) -> p m e", m=M),
        x_d[:, bs, :].rearrange("m p e -> p m e"),
    )
    xb = [xb_all[:, m * E : (m + 1) * E] for m in range(M)]
    # x with the folded bt2 bias, for the phase-2 residual (one 2x bf16 add)
    xbt_all = xbpool.tile([P, M * E], BF16, name="xbt", tag="xbt")
    nc.vector.tensor_add(xbt_all[:, :], xb_all[:, :], bt2_rep[:, :])
    # transpose each 128x128 block on the PE (via identity),
    # evacuating PSUM -> SBUF on the scalar engine
    xt = []
    xt8 = []
    for m in range(M):
        trm = trpsum.tile([P, EC * P], BF16, name=f"tr{m}", tag="tr")
        for c in range(EC):
            cs = slice(c * P, (c + 1) * P)
            nc.tensor.transpose(trm[:, cs], xb[m][:, cs], eye[:, :])
        xtm = xtpool.tile([P, EC * P], BF16, name=f"xt{m}", tag=f"xt{m}")
        nc.scalar.copy(xtm[:, :], trm[:, :])
        xt.append(xtm)
        if QK_FP8:
            # bf16 -> fp8 cast on the (idle) gpsimd engine, SBUF -> SBUF
            xtm8 = xtpool.tile([P, EC * P], FP8, name=f"xt8{m}", tag=f"xt8{m}")
            nc.gpsimd.tensor_copy(xtm8[:, :], xtm[:, :])
            xt8.append(xtm8)

    # qp|kp per modality into PSUM [128, 512]; keep all three PSUM banks
    # live so the tanh-input adds can read them directly (no evacuation).
    qkp = []
    for m in range(M):
        ps = qkpsum.tile([P, 2 * A], F32, name="qkp", tag="qkp")
        if QK_FP8:
            # DoubleRow fp8: two chunk-pairs per instruction.
            for c0 in (0, 2):
                nc.tensor.matmul(
                    ps[:, :],
                    xt8[m][:, c0 * P : (c0 + 2) * P].rearrange(
                        "p (two f) -> p two f", two=2
                    ),
                    wqk8[m][:, c0 * 2 * A : (c0 + 2) * 2 * A].rearrange(
                        "p (two f) -> p two f", two=2
                    ),
                    start=(c0 == 0),
                    stop=False,
                    perf_mode=mybir.MatmulPerfMode.DoubleRow,
                )
        else:
            for c in range(EC):
                nc.tensor.matmul(
                    ps[:, :],
                    xt[m][:, c * P : (c + 1) * P],
                    wqk[m][:, c * 2 * A : (c + 1) * 2 * A],
                    start=(c == 0),
                    stop=False,
                )
        nc.tensor.matmul(
            ps[:, :],
            ones_col[:1, :],
            bqk_row[m][:1, :],
            start=False,
            stop=True,
        )
        qkp.append(ps)

    tin = tinpool.tile([P, 2 * M * A], BF16, name="tin", tag="tin")
    if TIN_ON_POOL:
        # evacuate qp|kp to SBUF on Act, then tin adds on the gpsimd engine
        qkp_s = []
        for m in range(M):
            sb = xtpool.tile([P, 2 * A], BF16, name=f"qkps{m}", tag=f"qkps{m}")
            nc.scalar.copy(sb[:, :], qkp[m][:, :])
            qkp_s.append(sb)
        for idx, (q, k) in enumerate(pairs):
            nc.gpsimd.tensor_add(
                tin[:, idx * A : (idx + 1) * A],
                qkp_s[q][:, 0:A],
                qkp_s[k][:, A : 2 * A],
            )
    else:
        # tanh inputs straight from PSUM: tin[idx] = qp[q] + kp[k]
        for idx, (q, k) in enumerate(pairs):
            nc.vector.tensor_add(
                tin[:, idx * A : (idx + 1) * A],
                qkp[q][:, 0:A],
                qkp[k][:, A : 2 * A],
            )
    tth = ttpool.tile([P, 2 * M * A], BF16, name="tt", tag="tt")
    nc.scalar.activation(tth[:, :], tin[:, :], AF.Tanh)
    s_t = smpool.tile([P, 8], F32, name="scores", tag="scores")
    for idx, (q, k) in enumerate(pairs):
        tsc = tscpool.tile([P, A], BF16, name="tsc", tag="tsc")
        nc.vector.scalar_tensor_tensor(
            tsc[:, :],
            tth[:, idx * A : (idx + 1) * A],
            1.0,
            v_rep[q][:, :],
            AL.mult,
            AL.mult,
            accum_out=s_t[:, idx : idx + 1],
        )

    # alpha: a1 = sigmoid(s_first - s_second), a2 = 1 - a1
    d_t = smpool.tile([P, 4], F32, name="dsc", tag="dsc")
    nc.vector.tensor_sub(d_t[:, 0:M], s_t[:, 0:M], s_t[:, M : 2 * M])
    a1 = smpool.tile([P, 4], F32, name="a1", tag="a1")
    nc.scalar.activation(a1[:, 0:M], d_t[:, 0:M], AF.Sigmoid)
    a2 = smpool.tile([P, 4], F32, name="a2", tag="a2")
    nc.vector.tensor_scalar(
        a2[:, 0:M], a1[:, 0:M], -1.0, 1.0, AL.mult, AL.add
    )
    return {"bs": bs, "xbt": xbt_all, "xt": xt, "a1": a1, "a2": a2}


def _emit_phase2(
    nc, tc, st, fast_gb, pairs,
    out_d, wt, bt_row, ones_col, g_rep, b_rep,
    smpool, tmppool, attpool, sqpool, outpool, ypsum,
):
    bs, xbt, xt, a1, a2 = st["bs"], st["xbt"], st["xt"], st["a1"], st["a2"]

    # attended = (x+bt2) + a1*(y1 + bt1-bt2) + a2*y2  (bf16);
    # rows = per-q row sums.  Only the y1 group carries a rank-1 bias
    # (dbt = bt1-bt2); bt2 was folded into xbt in phase 1.
    attended = attpool.tile([P, M * E], BF16, name="attended", tag="attended")
    rows = smpool.tile([P, 4], F32, name="rows", tag="rows")
    for q in range(M):
        ys = []
        for j, k in enumerate((K_FIRST[q], K_SECOND[q])):
            ps = ypsum.tile([P, E], F32, name="y", tag="y")
            for c in range(EC):
                nc.tensor.matmul(
                    ps[:, :],
                    xt[k][:, c * P : (c + 1) * P],
                    wt[(q, k)][:, c * E : (c + 1) * E],
                    start=(c == 0),
                    stop=(j == 1 and c == EC - 1),
                )
            if j == 0:
                nc.tensor.matmul(
                    ps[:, :],
                    ones_col[:1, :],
                    bt_row[(q, k)][:1, :],
                    start=False,
                    stop=True,
                )
            ys.append(ps)
        tmp = tmppool.tile([P, E], F32, name="tmp", tag="tmp")
        nc.vector.scalar_tensor_tensor(
            tmp[:, :],
            ys[0][:, :],
            a1[:, q : q + 1],
            xbt[:, q * E : (q + 1) * E],
            AL.mult,
            AL.add,
        )
        nc.vector.scalar_tensor_tensor(
            attended[:, q * E : (q + 1) * E],
            ys[1][:, :],
            a2[:, q : q + 1],
            tmp[:, :],
            AL.mult,
            AL.add,
            accum_out=rows[:, q : q + 1],
        )

    # LayerNorm stats
    mu = smpool.tile([P, 1], F32, name="mu", tag="mu")
    nc.vector.tensor_reduce(
        mu[:, :], rows[:, 0:M], mybir.AxisListType.X, AL.add
    )
    nc.vector.tensor_scalar(
        mu[:, :], mu[:, :], 1.0 / (M * E), None, AL.mult
    )
    ssq = smpool.tile([P, 4], F32, name="ssq", tag="ssq")
    if SQ_ON_POOL:
        sqs = sqpool.tile([P, M * E], BF16, name="sq", tag="sq")
        for q in range(M):
            nc.gpsimd.tensor_mul(
                sqs[:, q * E : (q + 1) * E],
                attended[:, q * E : (q + 1) * E],
                attended[:, q * E : (q + 1) * E],
            )
        nc.gpsimd.tensor_reduce(
            ssq[:, 0:1], sqs[:, :], mybir.AxisListType.X, AL.add
        )
        ex2 = smpool.tile([P, 1], F32, name="ex2", tag="ex2")
        nc.vector.tensor_scalar(
            ex2[:, :], ssq[:, 0:1], 1.0 / (M * E), LN_EPS, AL.mult, AL.add
        )
    else:
        for q in range(M):
            sq = sqpool.tile([P, E], F32, name="sq", tag="sq")
            nc.scalar.activation(
                sq[:, :],
                attended[:, q * E : (q + 1) * E],
                AF.Square,
                accum_out=ssq[:, q : q + 1],
            )
        ex2 = smpool.tile([P, 1], F32, name="ex2", tag="ex2")
        nc.vector.tensor_reduce(
            ex2[:, :], ssq[:, 0:M], mybir.AxisListType.X, AL.add
        )
        nc.vector.tensor_scalar(
            ex2[:, :], ex2[:, :], 1.0 / (M * E), LN_EPS, AL.mult, AL.add
        )
    mu2 = smpool.tile([P, 1], F32, name="mu2", tag="mu2")
    nc.vector.tensor_mul(mu2[:, :], mu[:, :], mu[:, :])
    varp = smpool.tile([P, 1], F32, name="varp", tag="varp")
    nc.vector.tensor_sub(varp[:, :], ex2[:, :], mu2[:, :])

    # rstd = 1/sqrt(varp) via Heron on DVE only (2 Newton steps on sqrt,
    # then reciprocal).  s0=(1+v)/2; s' = (s + v/s)/2.  Keeps the Act
    # engine inside the sigmoid/tanh table set (no table reloads).
    sd = smpool.tile([P, 1], F32, name="sd0", tag="sd0")
    nc.vector.tensor_scalar(
        sd[:, :], varp[:, :], 0.5, 0.5, AL.mult, AL.add
    )
    for it in range(2):
        rc = smpool.tile([P, 1], F32, name=f"rc{it}", tag=f"rc{it}")
        nc.vector.reciprocal(rc[:, :], sd[:, :])
        sn = smpool.tile([P, 1], F32, name=f"sn{it}", tag=f"sn{it}")
        # sn = (rc * varp + sd) * 0.5  ==  (sd + v/sd)/2
        nc.vector.scalar_tensor_tensor(
            sn[:, :], rc[:, :], varp[:, 0:1], sd[:, :], AL.mult, AL.add
        )
        sd2 = smpool.tile([P, 1], F32, name=f"sd{it+1}", tag=f"sd{it+1}")
        nc.vector.tensor_scalar(sd2[:, :], sn[:, :], 0.5, None, AL.mult)
        sd = sd2
    rstd = smpool.tile([P, 1], F32, name="rstd", tag="rstd")
    nc.vector.reciprocal(rstd[:, :], sd[:, :])

    out_dt = BF16 if (OUT_BF16 and fast_gb) else F32
    out_t = outpool.tile([P, M * E], out_dt, name="out", tag="out")
    if fast_gb:
        if OUT_ON_ACT:
            mb = smpool.tile([P, 1], F32, name="mb", tag="mb")
            nc.vector.tensor_mul(mb[:, :], mu[:, :], rstd[:, :])
            nc.vector.tensor_scalar(mb[:, :], mb[:, :], -1.0, None, AL.mult)
            nc.scalar.activation(
                out_t[:, :], attended[:, :], AF.Identity,
                bias=mb[:, 0:1], scale=rstd[:, 0:1],
            )
        else:
            nc.vector.tensor_scalar(
                out_t[:, :],
                attended[:, :],
                mu[:, 0:1],
                rstd[:, 0:1],
                AL.subtract,
                AL.mult,
            )
    else:
        za = outpool.tile([P, M * E], F32, name="za", tag="za")
        nc.vector.tensor_scalar(
            za[:, :],
            attended[:, :],
            mu[:, 0:1],
            rstd[:, 0:1],
            AL.subtract,
            AL.mult,
        )
        gz = outpool.tile([P, M * E], F32, name="gz", tag="gz")
        nc.vector.tensor_mul(gz[:, :], za[:, :], g_rep[:, :])
        nc.vector.tensor_add(out_t[:, :], gz[:, :], b_rep[:, :])
    if out_dt == F32:
        nc.sync.dma_start(out_d[bs, :], out_t[:, :])
    else:
        # bf16 -> f32 upcast store must be initiated from the gpsimd queue
        nc.gpsimd.dma_start(out_d[bs, :], out_t[:, :])


_PROGRAM_CACHE: dict = {}


def _get_program(bc: int, fast_gb: bool, reps: int = 1) -> bass.Bass:
    key = (bc, fast_gb, reps)
    if key not in _PROGRAM_CACHE:
        _PROGRAM_CACHE[key] = _build(bc, fast_gb, reps)
    return _PROGRAM_CACHE[key]


def _prep_shared(ins) -> dict:
    """Host-side bf16 pre-arrangement of the (small) parameters."""
    import ml_dtypes

    BFD = ml_dtypes.bfloat16
    F8D = ml_dtypes.float8_e4m3fn
    pairs = [(q, K_FIRST[q]) for q in range(M)] + [
        (q, K_SECOND[q]) for q in range(M)
    ]
    Wq, Wk, Wt = ins["Wq"], ins["Wk"], ins["Wt"]
    wqk = np.empty((M, EC, P, 2 * A), BFD)
    for m in range(M):
        for c in range(EC):
            wqk[m, c, :, :A] = Wq[m, c * P : (c + 1) * P, :]
            wqk[m, c, :, A:] = Wk[m, c * P : (c + 1) * P, :]
    wtb = np.empty((2 * M, EC, P, E), BFD)
    for j, (q, k) in enumerate(pairs):
        for c in range(EC):
            wtb[j, c] = Wt[q, k, c * P : (c + 1) * P, :]
    bqk = np.concatenate([ins["bq"], ins["bk"]], axis=1).astype(BFD)
    # rows j<M: dbt[q] = bt[q,k1]-bt[q,k2]; rows j>=M: bt2[q] = bt[q,k2]
    btr = np.stack(
        [
            ins["bt"][q, K_FIRST[q]] - ins["bt"][q, K_SECOND[q]]
            for q in range(M)
        ]
        + [ins["bt"][q, K_SECOND[q]] for q in range(M)]
    ).astype(BFD)
    shared = {
        "Wqk_bf": wqk,
        "Wt_bf": wtb,
        "bqk_bf": np.ascontiguousarray(bqk),
        "btr_bf": np.ascontiguousarray(btr),
        "v_bf": ins["v"].astype(BFD),
        "eye_bf": np.eye(P, dtype=BFD),
        "gamma": ins["gamma"],
        "beta": ins["beta"],
    }
    if QK_FP8:
        wqk8 = np.empty((M, EC, P, 2 * A), F8D)
        for m in range(M):
            for c in range(EC):
                wqk8[m, c, :, :A] = Wq[m, c * P : (c + 1) * P, :]
                wqk8[m, c, :, A:] = Wk[m, c * P : (c + 1) * P, :]
        shared["Wqk_f8"] = wqk8
    return shared


def kernel(**inputs) -> np.ndarray:
    ins = {
        k: np.ascontiguousarray(np.asarray(v, dtype=np.float32))
        for k, v in inputs.items()
    }
    x = ins["x"]
    assert x.shape == (M, B_FULL, E), x.shape
    fast_gb = bool(np.all(ins["gamma"] == 1.0) and np.all(ins["beta"] == 0.0))
    nc = _get_program(BC, fast_gb)

    shared = _prep_shared(ins)
    in_maps = []
    for i in range(N_CORES):
        m = dict(shared)
        m["x"] = np.ascontiguousarray(x[:, i * BC : (i + 1) * BC, :])
        in_maps.append(m)

    res = run_bass_kernel_spmd(nc, in_maps, core_ids=list(range(N_CORES)))
    out = np.concatenate([res.results[i]["out"] for i in range(N_CORES)], axis=0)
    return out


if __name__ == "__main__":
    rng = np.random.default_rng(0)
    ins = {
        "x": rng.standard_normal((M, B_FULL, E), dtype=np.float32),
        "Wq": (rng.standard_normal((M, E, A)) / np.sqrt(E)).astype(np.float32),
        "bq": (rng.standard_normal((M, A)) / np.sqrt(E)).astype(np.float32),
        "Wk": (rng.standard_normal((M, E, A)) / np.sqrt(E)).astype(np.float32),
        "bk": (rng.standard_normal((M, A)) / np.sqrt(E)).astype(np.float32),
        "v": (rng.standard_normal((M, A)) / np.sqrt(A)).astype(np.float32),
        "Wt": (rng.standard_normal((M, M, E, E)) / np.sqrt(E)).astype(np.float32),
        "bt": (rng.standard_normal((M, M, E)) / np.sqrt(E)).astype(np.float32),
        "gamma": np.ones((M * E,), np.float32),
        "beta": np.zeros((M * E,), np.float32),
    }
    out = kernel(**ins)
    print("out", out.shape, out.dtype)



# revision 16
# speedup vs baseline: 2.1445x; 2.1445x over previous
"""Trainium2 Bass kernel for CrossModalAttention.

Reference computation (per batch row b, modalities q,k in {0,1,2}):
  qp[m] = x[m] @ Wq[m] + bq[m];  kp[m] = x[m] @ Wk[m] + bk[m]
  scores[q,k] = v[q] . tanh(qp[q] + kp[k])          (k != q)
  alpha = softmax over k (2 off-diagonal entries per q)
  att[q] = sum_k alpha[q,k] * (x[k] @ Wt[q,k] + bt[q,k])
  fused  = LayerNorm(concat_m(x[m] + att[m]); gamma, beta)

Pure data parallel over the batch across 8 NeuronCores (8192 rows per
core), parameters replicated; per core, batch tiles of 128 rows.

Design (v2, rebuilt around the TimelineSim cost model):
  - x^T via DMA-crossbar transposes (dma_start_transpose on the SP
    queue) instead of PE transposes + Act evacuation.
  - fp8 copy of x^T produced by a single gpsimd cast-DMA (SBUF->SBUF).
  - qk projections are pair-fused: for each query modality q one PSUM
    bank [128, 512] accumulates qp[q]+kp[k1] | qp[q]+kp[k2] directly
    (Wq duplicated host-side so the q-side matmul covers both halves),
    in fp8 DoubleRow.  The tanh reads the bank straight from PSUM; no
    PSUM evacuation and no separate tanh-input adds.
  - all biases are rank-1 DoubleRow fp8 matmuls ([ones|zeros] stationary
    trick), 2x cheaper than bf16 rank-1s.
  - att: per q two PSUM banks y1 (k1, fp8 DoubleRow or split-K) and
    y2 (k2, bf16); combine attended = a1*y1 + (a2*y2 + x) as two DVE
    scalar_tensor_tensor ops with the row-sum accumulated for free.
  - LN: squares on Act (Square + accumulator), rstd via DVE-only
    Newton iteration (no Sqrt table reload), normalize as a single DVE
    tensor_scalar which hits the 4x_2p perf mode (all-bf16 SBUF).
  - output stored as bf16 (the data only ever had bf16 precision) on
    the SP HWDGE queue; host upcasts to f32.
"""

import json

import numpy as np

import concourse.bass as bass
import concourse.bass2jax as bass2jax
import concourse.bass_utils as bass_utils
import concourse.mybir as mybir
import concourse.tile as tile
from concourse.bass_utils import run_bass_kernel_spmd

M, E, A = 3, 512, 256
B_FULL = 65536
N_CORES = 8
BC = B_FULL // N_CORES  # 8192 rows per core
P = 128
EC = E // P  # 4 contraction chunks
LN_EPS = 1e-5

F32 = mybir.dt.float32
BF16 = mybir.dt.bfloat16
FP8 = mybir.dt.float8e4
AL = mybir.AluOpType
AF = mybir.ActivationFunctionType
DR = mybir.MatmulPerfMode.DoubleRow

# --- tuning flags -----------------------------------------------------------
# Precision mode of the a1-weighted Wt matmul (y1):
#   'fp8'  - both K-chunk pairs in fp8 DoubleRow (fastest, ~1.7e-2 rel)
#   'half' - chunks 0,1 fp8 DR, chunks 2,3 bf16 (safer, ~1.5e-2 rel)
#   'bf16' - all bf16 (safest, ~1e-2 rel)
Y1_MODE = "half"
SCORES_ON_POOL = False  # per-pair score reductions on gpsimd vs DVE
# pool buffer counts (sweepable)
BUFS = dict(xb=8, xt=6, xt8=6, tth=5, tsc=4, small=8, tmp=4, att=4, sq=4,
            outp=6, tin=3, y=5)
PIPE_STAGES = 2  # 2: load | rest+1; 3: load | transpose | rest
STORE_DEFER = 3  # extra iterations before issuing each output store

# For query modality q the two keys, in a fixed order.
K_FIRST = [1, 0, 0]
K_SECOND = [2, 2, 1]

# ---------------------------------------------------------------------------
# The walrus build in this container rejects instructions carrying more than
# one semaphore wait.  Legalize the serialized BIR: move excess waits onto
# NoOp instructions inserted just before the offender on the same engine.
# ---------------------------------------------------------------------------
_MAX_WAITS = 1
_REAL_ENGINES = {"PE", "DVE", "Activation", "Pool", "SP"}


def _legalize_waits(bir_json) -> bytes:
    d = json.loads(bir_json)
    for f in d.get("functions", []):
        for b in f.get("blocks", []):
            insts = b.get("instructions", [])
            out = []
            for inst in insts:
                si = inst.get("sync_info")
                waits = (si or {}).get("on_wait") or []
                if len(waits) > _MAX_WAITS and inst.get("engine") in _REAL_ENGINES:
                    extra = waits[: len(waits) - _MAX_WAITS]
                    si["on_wait"] = waits[len(waits) - _MAX_WAITS :]
                    for j, w in enumerate(extra):
                        out.append(
                            {
                                "debug": inst.get("debug", 0),
                                "engine": inst["engine"],
                                "ins": [],
                                "name": f"{inst['name']}-ws{j}",
                                "opcode": "NoOp",
                                "outs": [],
                                "sync_info": {"on_update": [], "on_wait": [w]},
                            }
                        )
                out.append(inst)
            b["instructions"] = out
    return json.dumps(d).encode()


_orig_compile_bir_kernel = bass_utils.compile_bir_kernel


def _patched_compile_bir_kernel(bir_json, tmpdir, neff_name="file.neff"):
    return _orig_compile_bir_kernel(_legalize_waits(bir_json), tmpdir, neff_name)


if bass_utils.compile_bir_kernel is not _patched_compile_bir_kernel:
    bass_utils.compile_bir_kernel = _patched_compile_bir_kernel
    bass2jax.compile_bir_kernel = _patched_compile_bir_kernel


def _dr(ap):
    """Rearrange a [p, 2*F] slice into DoubleRow [p, 2, F] form."""
    return ap.rearrange("p (two f) -> p two f", two=2)


def _build(bc: int, fast_gb: bool, reps: int = 1) -> bass.Bass:
    nt = bc // P
    nc = bass.Bass()

    x_d = nc.dram_tensor("x", [M, bc, E], F32, kind="ExternalInput")
    g_d = nc.dram_tensor("gamma", [M * E], F32, kind="ExternalInput")
    be_d = nc.dram_tensor("beta", [M * E], F32, kind="ExternalInput")
    # pre-arranged parameters (prepared host-side in kernel()):
    wqq8_d = nc.dram_tensor("Wqq8", [M, EC, P, 2 * A], FP8, kind="ExternalInput")
    wk8_d = nc.dram_tensor("Wk8", [M, EC, P, A], FP8, kind="ExternalInput")
    wt1_dt = FP8 if Y1_MODE in ("fp8", "half") else BF16
    wt18_d = nc.dram_tensor("Wt18", [M, EC, P, E], wt1_dt, kind="ExternalInput")
    if Y1_MODE == "half":
        wt1b_d = nc.dram_tensor("Wt1b", [M, 2, P, E], BF16, kind="ExternalInput")
    wt2_d = nc.dram_tensor("Wt2", [M, EC, P, E], BF16, kind="ExternalInput")
    bqk8_d = nc.dram_tensor("bqk8", [M, 4 * A], FP8, kind="ExternalInput")
    bt18_d = nc.dram_tensor("bt18", [M, 2 * E], FP8, kind="ExternalInput")
    bt28_d = nc.dram_tensor("bt28", [M, 2 * E], FP8, kind="ExternalInput")
    vb_d = nc.dram_tensor("v_bf", [M, A], BF16, kind="ExternalInput")
    out_d = nc.dram_tensor("out", [bc, M * E], BF16, kind="ExternalOutput")

    with tile.TileContext(nc) as tc:
        with (
            tc.tile_pool(name="const", bufs=1) as cpool,
            tc.tile_pool(name="xb", bufs=BUFS["xb"]) as xbpool,
            tc.tile_pool(name="xt", bufs=BUFS["xt"]) as xtpool,
            tc.tile_pool(name="xt8", bufs=BUFS["xt8"]) as xt8pool,
            tc.tile_pool(name="tth", bufs=BUFS["tth"]) as tthpool,
            tc.tile_pool(name="tsc", bufs=BUFS["tsc"]) as tscpool,
            tc.tile_pool(name="small", bufs=BUFS["small"]) as smpool,
            tc.tile_pool(name="tmp", bufs=BUFS["tmp"]) as tmppool,
            tc.tile_pool(name="att", bufs=BUFS["att"]) as attpool,
            tc.tile_pool(name="sq", bufs=BUFS["sq"]) as sqpool,
            tc.tile_pool(name="outp", bufs=BUFS["outp"]) as outpool,
            tc.tile_pool(name="tin_ps", bufs=BUFS["tin"], space=bass.MemorySpace.PSUM) as tinps,
            tc.tile_pool(name="y_ps", bufs=BUFS["y"], space=bass.MemorySpace.PSUM) as yps,
        ):
            # ---- resident parameters ----
            wqq8 = [
                cpool.tile([P, EC * 2 * A], FP8, name=f"wqq8{m}", tag=f"wqq8{m}")
                for m in range(M)
            ]
            wk8 = [
                cpool.tile([P, EC * A], FP8, name=f"wk8{m}", tag=f"wk8{m}")
                for m in range(M)
            ]
            wt1 = [
                cpool.tile([P, EC * E], wt1_dt, name=f"wt1{q}", tag=f"wt1{q}")
                for q in range(M)
            ]
            wt2 = [
                cpool.tile([P, EC * E], BF16, name=f"wt2{q}", tag=f"wt2{q}")
                for q in range(M)
            ]
            for m in range(M):
                nc.sync.dma_start(
                    wqq8[m][:, :].rearrange("p (c a) -> p c a", c=EC),
                    wqq8_d[m].rearrange("c p a -> p c a"),
                )
                nc.sync.dma_start(
                    wk8[m][:, :].rearrange("p (c a) -> p c a", c=EC),
                    wk8_d[m].rearrange("c p a -> p c a"),
                )
                nc.sync.dma_start(
                    wt1[m][:, :].rearrange("p (c e) -> p c e", c=EC),
                    wt18_d[m].rearrange("c p e -> p c e"),
                )
                nc.sync.dma_start(
                    wt2[m][:, :].rearrange("p (c e) -> p c e", c=EC),
                    wt2_d[m].rearrange("c p e -> p c e"),
                )
            if Y1_MODE == "half":
                wt1b = [
                    cpool.tile([P, 2 * E], BF16, name=f"wt1b{q}", tag=f"wt1b{q}")
                    for q in range(M)
                ]
                for m in range(M):
                    nc.sync.dma_start(
                        wt1b[m][:, :].rearrange("p (c e) -> p c e", c=2),
                        wt1b_d[m].rearrange("c p e -> p c e"),
                    )
            bqk8 = [
                cpool.tile([1, 4 * A], FP8, name=f"bqk8{m}", tag=f"bqk8{m}")
                for m in range(M)
            ]
            bt18 = [
                cpool.tile([1, 2 * E], FP8, name=f"bt18{m}", tag=f"bt18{m}")
                for m in range(M)
            ]
            bt28 = [
                cpool.tile([1, 2 * E], FP8, name=f"bt28{m}", tag=f"bt28{m}")
                for m in range(M)
            ]
            for m in range(M):
                nc.sync.dma_start(
                    bqk8[m][:1, :], bqk8_d[m, :].rearrange("(o a) -> o a", o=1)
                )
                nc.sync.dma_start(
                    bt18[m][:1, :], bt18_d[m, :].rearrange("(o e) -> o e", o=1)
                )
                nc.sync.dma_start(
                    bt28[m][:1, :], bt28_d[m, :].rearrange("(o e) -> o e", o=1)
                )
            # [ones(128) | zeros(128)] fp8 stationary for rank-1 DR biases
            ones8 = cpool.tile([1, 2 * P], FP8, name="ones8", tag="ones8")
            nc.vector.memset(ones8[:1, 0:P], 1.0)
            nc.vector.memset(ones8[:1, P : 2 * P], 0.0)
            ones_bf = cpool.tile([1, P], BF16, name="onesb", tag="onesb")
            nc.vector.memset(ones_bf[:1, :], 1.0)

            # v replicated across partitions via rank-1 matmul
            v_rep = []
            for q in range(M):
                row = cpool.tile([1, A], BF16, name=f"vrow{q}", tag=f"vrow{q}")
                nc.sync.dma_start(
                    row[:1, :], vb_d[q, :].rearrange("(o a) -> o a", o=1)
                )
                ps = yps.tile([P, 2 * A], F32, name=f"vbc{q}", tag="y")
                nc.tensor.matmul(ps[:, 0:A], ones_bf[:1, :], row[:1, :])
                rep = cpool.tile([P, A], BF16, name=f"vrep{q}", tag=f"vrep{q}")
                nc.scalar.copy(rep[:, :], ps[:, 0:A])
                v_rep.append(rep)

            # gamma/beta replicated (general path only)
            g_rep = b_rep = None
            if not fast_gb:
                ones_f32 = cpool.tile([1, P], F32, name="onesf", tag="onesf")
                nc.vector.memset(ones_f32[:1, :], 1.0)
                grow = cpool.tile([1, M * E], F32, name="grow", tag="grow")
                nc.gpsimd.dma_start(grow[:1, :], g_d[:].rearrange("(o e) -> o e", o=1))
                brow = cpool.tile([1, M * E], F32, name="brow", tag="brow")
                nc.gpsimd.dma_start(brow[:1, :], be_d[:].rearrange("(o e) -> o e", o=1))
                g_rep = cpool.tile([P, M * E], F32, name="grep", tag="grep")
                b_rep = cpool.tile([P, M * E], F32, name="brep", tag="brep")
                for src, dst in ((grow, g_rep), (brow, b_rep)):
                    for c in range(M):
                        cs = slice(c * E, (c + 1) * E)
                        ps = yps.tile([P, E], F32, name=f"gbc{c}", tag="y")
                        nc.tensor.matmul(ps[:, :], ones_f32[:1, :], src[:1, cs])
                        nc.vector.tensor_copy(dst[:, cs], ps[:, :])

            consts = dict(
                wqq8=wqq8, wk8=wk8, wt1=wt1, wt2=wt2,
                wt1b=wt1b if Y1_MODE == "half" else None,
                bqk8=bqk8, bt18=bt18, bt28=bt28, ones8=ones8,
                v_rep=v_rep, g_rep=g_rep, b_rep=b_rep,
            )
            pools = dict(
                xb=xbpool, xt=xtpool, xt8=xt8pool, tth=tthpool, tsc=tscpool,
                sm=smpool, tmp=tmppool, att=attpool, sq=sqpool, out=outpool,
                tin=tinps, y=yps,
            )

            import contextlib

            rep_ctx = tc.For_i(0, reps, 1) if reps > 1 else contextlib.nullcontext()
            with rep_ctx:
                # 3-stage software pipeline: per iteration i emit
                #   A(i+2): x load            (Pool DMA)
                #   B(i+1): transpose, fp8 cast, qk matmuls, tanh
                #   C(i):   scores, alpha, Wt matmuls, combine, LN, store
                # so no engine stream ever waits on a same-iteration long
                # dependency chain.
                stA: dict = {}
                stB: dict = {}
                pending_store: list = []
                for i in range(nt + PIPE_STAGES):
                    if i < nt:
                        stA[i] = _emit_load(nc, i, x_d, pools)
                    if PIPE_STAGES == 3 and 0 <= i - 1 < nt:
                        _emit_transpose(nc, stA[i - 1], pools)
                    b = i - (PIPE_STAGES - 1)
                    if 0 <= b < nt:
                        stB[b] = _emit_phase1(nc, stA.pop(b), consts, pools)
                    if 0 <= i - PIPE_STAGES < nt:
                        st = _emit_phase2(
                            nc, stB.pop(i - PIPE_STAGES), fast_gb, out_d,
                            consts, pools
                        )
                        # defer stores so their waits are satisfied before
                        # the SP SEQ reaches them (a waiting DMA blocks its
                        # queue's sequencer, which would stall transposes)
                        pending_store.append(st)
                        if len(pending_store) > STORE_DEFER:
                            bs0, t0 = pending_store.pop(0)
                            nc.sync.dma_start(out_d[bs0, :], t0[:, :])
                for bs0, t0 in pending_store:
                    nc.sync.dma_start(out_d[bs0, :], t0[:, :])

    return nc


def _emit_load(nc, ti, x_d, PL):
    bs = slice(ti * P, (ti + 1) * P)
    # x for all modalities, cast f32 -> bf16 in one gpsimd DMA:
    # xb[p, m*E + e] = x[m, bs.start+p, e]
    xb = PL["xb"].tile([P, M * E], BF16, name="xb", tag="xb")
    nc.gpsimd.dma_start(
        xb[:, :].rearrange("p (m e) -> p m e", m=M),
        x_d[:, bs, :].rearrange("m p e -> p m e"),
    )
    return {"bs": bs, "xb": xb}


def _emit_transpose(nc, st, PL):
    xb = st["xb"]
    # x^T per modality via DMA crossbar transpose:
    # xt[p, m*E + c*P + b] = x^T chunk: = xb[b, m*E + c*P + p]
    xt = PL["xt"].tile([P, M * E], BF16, name="xt", tag="xt")
    for m in range(M):
        nc.sync.dma_start_transpose(
            xt[:, m * E : (m + 1) * E].rearrange("p (c b) -> p c b", c=EC),
            xb[:, m * E : (m + 1) * E],
        )
    st["xt"] = xt
    return st


def _emit_phase1(nc, st, C, PL):
    if "xt" not in st:
        _emit_transpose(nc, st, PL)
    xb, xt = st["xb"], st["xt"]
    # fp8 copy of x^T in one gpsimd cast-DMA (SBUF -> SBUF)
    xt8 = PL["xt8"].tile([P, M * E], FP8, name="xt8", tag="xt8")
    nc.gpsimd.dma_start(xt8[:, :], xt[:, :])

    def xt_chunk(m, c):  # bf16 x^T chunk [128, 128]
        return xt[:, m * E + c * P : m * E + (c + 1) * P]

    def xt8_pair(m, c0):  # fp8 x^T chunk-pair [128, 2, 128]
        return _dr(xt8[:, m * E + c0 * P : m * E + (c0 + 2) * P])

    # pair-fused qk projections: bank q = [qp[q]+kp[k1]+b | qp[q]+kp[k2]+b]
    tin = []
    for q in range(M):
        k1, k2 = K_FIRST[q], K_SECOND[q]
        ps = PL["tin"].tile([P, 2 * A], F32, name="tin", tag="tin")
        for i, c0 in enumerate((0, 2)):
            nc.tensor.matmul(
                ps[:, :],
                xt8_pair(q, c0),
                _dr(C["wqq8"][q][:, c0 * 2 * A : (c0 + 2) * 2 * A]),
                start=(i == 0),
                stop=False,
                perf_mode=DR,
            )
        for half, k in enumerate((k1, k2)):
            hs = slice(half * A, (half + 1) * A)
            for c0 in (0, 2):
                nc.tensor.matmul(
                    ps[:, hs],
                    xt8_pair(k, c0),
                    _dr(C["wk8"][k][:, c0 * A : (c0 + 2) * A]),
                    start=False,
                    stop=False,
                    perf_mode=DR,
                )
        nc.tensor.matmul(
            ps[:, :],
            _dr(C["ones8"][:1, :]),
            _dr(C["bqk8"][q][:1, :]),
            start=False,
            stop=True,
            perf_mode=DR,
        )
        tin.append(ps)

    # tanh straight from PSUM, one Act op per bank
    tth = PL["tth"].tile([P, 2 * M * A], BF16, name="tth", tag="tth")
    for q in range(M):
        nc.scalar.activation(
            tth[:, q * 2 * A : (q + 1) * 2 * A], tin[q][:, :], AF.Tanh
        )


    # scores: s[q] (half 0, key k1) and s[3+q] (half 1, key k2)
    s_t = PL["sm"].tile([P, 8], F32, name="scores", tag="scores")
    eng = nc.gpsimd if SCORES_ON_POOL else nc.vector
    for q in range(M):
        for half in range(2):
            tsc = PL["tsc"].tile([P, A], BF16, name="tsc", tag="tsc")
            eng.scalar_tensor_tensor(
                tsc[:, :],
                tth[:, (2 * q + half) * A : (2 * q + half + 1) * A],
                1.0,
                C["v_rep"][q][:, :],
                AL.mult,
                AL.mult,
                accum_out=s_t[:, 3 * half + q : 3 * half + q + 1],
            )

    # alpha: a1 = sigmoid(s1 - s2), a2 = 1 - a1
    d_t = PL["sm"].tile([P, 4], F32, name="dsc", tag="dsc")
    nc.vector.tensor_sub(d_t[:, 0:M], s_t[:, 0:M], s_t[:, M : 2 * M])
    a1 = PL["sm"].tile([P, 4], F32, name="a1", tag="a1")
    nc.scalar.activation(a1[:, 0:M], d_t[:, 0:M], AF.Sigmoid)
    a2 = PL["sm"].tile([P, 4], F32, name="a2", tag="a2")
    nc.vector.tensor_scalar(a2[:, 0:M], a1[:, 0:M], -1.0, 1.0, AL.mult, AL.add)

    return {"bs": st["bs"], "xb": xb, "xt": xt, "xt8": xt8,
            "a1": a1, "a2": a2, "xt_chunk": xt_chunk, "xt8_pair": xt8_pair}


def _emit_phase2(nc, st, fast_gb, out_d, C, PL):
    bs, xb, a1, a2 = st["bs"], st["xb"], st["a1"], st["a2"]
    xt_chunk, xt8_pair = st["xt_chunk"], st["xt8_pair"]

    attended = PL["att"].tile([P, M * E], BF16, name="attended", tag="attended")
    rows = PL["sm"].tile([P, 4], F32, name="rows", tag="rows")
    for q in range(M):
        k1, k2 = K_FIRST[q], K_SECOND[q]
        # y1 = x[k1] @ Wt[q,k1] + bt[q,k1]
        y1 = PL["y"].tile([P, E], F32, name="y1", tag="y")
        if Y1_MODE == "fp8":
            for i, c0 in enumerate((0, 2)):
                nc.tensor.matmul(
                    y1[:, :], xt8_pair(k1, c0),
                    _dr(C["wt1"][q][:, c0 * E : (c0 + 2) * E]),
                    start=(i == 0), stop=False, perf_mode=DR,
                )
        elif Y1_MODE == "half":
            nc.tensor.matmul(
                y1[:, :], xt8_pair(k1, 0), _dr(C["wt1"][q][:, 0 : 2 * E]),
                start=True, stop=False, perf_mode=DR,
            )
            for c in (2, 3):
                nc.tensor.matmul(
                    y1[:, :], xt_chunk(k1, c),
                    C["wt1b"][q][:, (c - 2) * E : (c - 1) * E],
                    start=False, stop=False,
                )
        else:
            for c in range(EC):
                nc.tensor.matmul(
                    y1[:, :], xt_chunk(k1, c),
                    C["wt1"][q][:, c * E : (c + 1) * E],
                    start=(c == 0), stop=False,
                )
        nc.tensor.matmul(
            y1[:, :], _dr(C["ones8"][:1, :]), _dr(C["bt18"][q][:1, :]),
            start=False, stop=True, perf_mode=DR,
        )
        # y2 = x[k2] @ Wt[q,k2] + bt[q,k2]  (bf16)
        y2 = PL["y"].tile([P, E], F32, name="y2", tag="y")
        for c in range(EC):
            nc.tensor.matmul(
                y2[:, :], xt_chunk(k2, c), C["wt2"][q][:, c * E : (c + 1) * E],
                start=(c == 0), stop=False,
            )
        nc.tensor.matmul(
            y2[:, :], _dr(C["ones8"][:1, :]), _dr(C["bt28"][q][:1, :]),
            start=False, stop=True, perf_mode=DR,
        )
        # attended_q = a1*y1 + (a2*y2 + x_q), row-sum accumulated
        qs = slice(q * E, (q + 1) * E)
        tmp = PL["tmp"].tile([P, E], BF16, name="tmp", tag="tmp")
        nc.vector.scalar_tensor_tensor(
            tmp[:, :], y2[:, :], a2[:, q : q + 1], xb[:, qs], AL.mult, AL.add
        )
        nc.vector.scalar_tensor_tensor(
            attended[:, qs], y1[:, :], a1[:, q : q + 1], tmp[:, :],
            AL.mult, AL.add, accum_out=rows[:, q : q + 1],
        )

    # LayerNorm stats: mean from rows, E[x^2] from Act Square accumulators
    mu = PL["sm"].tile([P, 1], F32, name="mu", tag="mu")
    nc.vector.tensor_reduce(mu[:, :], rows[:, 0:M], mybir.AxisListType.X, AL.add)
    nc.vector.tensor_scalar(mu[:, :], mu[:, :], 1.0 / (M * E), None, AL.mult)
    ssq = PL["sm"].tile([P, 4], F32, name="ssq", tag="ssq")
    for q in range(M):
        sq = PL["sq"].tile([P, E], BF16, name="sq", tag="sq")
        nc.scalar.activation(
            sq[:, :], attended[:, q * E : (q + 1) * E], AF.Square,
            accum_out=ssq[:, q : q + 1],
        )
    ex2 = PL["sm"].tile([P, 1], F32, name="ex2", tag="ex2")
    nc.vector.tensor_reduce(ex2[:, :], ssq[:, 0:M], mybir.AxisListType.X, AL.add)
    nc.vector.tensor_scalar(
        ex2[:, :], ex2[:, :], 1.0 / (M * E), LN_EPS, AL.mult, AL.add
    )
    mu2 = PL["sm"].tile([P, 1], F32, name="mu2", tag="mu2")
    nc.vector.tensor_mul(mu2[:, :], mu[:, :], mu[:, :])
    varp = PL["sm"].tile([P, 1], F32, name="varp", tag="varp")
    nc.vector.tensor_sub(varp[:, :], ex2[:, :], mu2[:, :])

    # rstd = 1/sqrt(varp) via Heron on DVE only (keeps Act in the
    # tanh/sigmoid/square table set)
    sd = PL["sm"].tile([P, 1], F32, name="sd0", tag="sd0")
    nc.vector.tensor_scalar(sd[:, :], varp[:, :], 0.5, 0.5, AL.mult, AL.add)
    for it in range(2):
        rc = PL["sm"].tile([P, 1], F32, name=f"rc{it}", tag=f"rc{it}")
        nc.vector.reciprocal(rc[:, :], sd[:, :])
        sn = PL["sm"].tile([P, 1], F32, name=f"sn{it}", tag=f"sn{it}")
        nc.vector.scalar_tensor_tensor(
            sn[:, :], rc[:, :], varp[:, 0:1], sd[:, :], AL.mult, AL.add
        )
        sd2 = PL["sm"].tile([P, 1], F32, name=f"sd{it+1}", tag=f"sd{it+1}")
        nc.vector.tensor_scalar(sd2[:, :], sn[:, :], 0.5, None, AL.mult)
        sd = sd2
    rstd = PL["sm"].tile([P, 1], F32, name="rstd", tag="rstd")
    nc.vector.reciprocal(rstd[:, :], sd[:, :])

    out_t = PL["out"].tile([P, M * E], BF16, name="out", tag="out")
    # (attended - mu) * rstd: all-bf16 SBUF tensor_scalar -> 4x DVE mode
    nc.vector.tensor_scalar(
        out_t[:, :], attended[:, :], mu[:, 0:1], rstd[:, 0:1],
        AL.subtract, AL.mult,
    )
    if not fast_gb:
        gz = PL["out"].tile([P, M * E], F32, name="gz", tag="gz")
        nc.vector.tensor_mul(gz[:, :], out_t[:, :], C["g_rep"][:, :])
        nc.vector.tensor_add(gz[:, :], gz[:, :], C["b_rep"][:, :])
        nc.vector.tensor_copy(out_t[:, :], gz[:, :])
    return (bs, out_t)


_PROGRAM_CACHE: dict = {}


def _get_program(bc: int, fast_gb: bool, reps: int = 1) -> bass.Bass:
    key = (bc, fast_gb, reps)
    if key not in _PROGRAM_CACHE:
        _PROGRAM_CACHE[key] = _build(bc, fast_gb, reps)
    return _PROGRAM_CACHE[key]


def _prep_shared(ins) -> dict:
    """Host-side pre-arrangement of the (small) parameters."""
    import ml_dtypes

    BFD = ml_dtypes.bfloat16
    F8D = ml_dtypes.float8_e4m3fn
    Wq, Wk, Wt = ins["Wq"], ins["Wk"], ins["Wt"]
    bq, bk, bt = ins["bq"], ins["bk"], ins["bt"]

    # Wqq8[m, c, p, :] = [Wq[m][c*128+p] | Wq[m][c*128+p]]
    wq_c = Wq.reshape(M, EC, P, A)
    wqq8 = np.concatenate([wq_c, wq_c], axis=3).astype(F8D)
    wk8 = Wk.reshape(M, EC, P, A).astype(F8D)

    wt1f = np.stack([Wt[q, K_FIRST[q]] for q in range(M)])  # [M, E, E]
    wt2f = np.stack([Wt[q, K_SECOND[q]] for q in range(M)])
    wt1_dtype = F8D if Y1_MODE in ("fp8", "half") else BFD
    wt18 = wt1f.reshape(M, EC, P, E).astype(wt1_dtype)
    wt2 = wt2f.reshape(M, EC, P, E).astype(BFD)

    # biases: [bias | zeros] halves for the rank-1 DoubleRow trick
    z_a = np.zeros((M, 2 * A), np.float32)
    bqk = np.stack(
        [
            np.concatenate([bq[q] + bk[K_FIRST[q]], bq[q] + bk[K_SECOND[q]]])
            for q in range(M)
        ]
    )
    bqk8 = np.concatenate([bqk, z_a], axis=1).astype(F8D)
    z_e = np.zeros((M, E), np.float32)
    bt18 = np.concatenate(
        [np.stack([bt[q, K_FIRST[q]] for q in range(M)]), z_e], axis=1
    ).astype(F8D)
    bt28 = np.concatenate(
        [np.stack([bt[q, K_SECOND[q]] for q in range(M)]), z_e], axis=1
    ).astype(F8D)

    shared = {
        "Wqq8": wqq8,
        "Wk8": wk8,
        "Wt18": np.ascontiguousarray(wt18),
        "Wt2": np.ascontiguousarray(wt2),
        "bqk8": np.ascontiguousarray(bqk8),
        "bt18": np.ascontiguousarray(bt18),
        "bt28": np.ascontiguousarray(bt28),
        "v_bf": ins["v"].astype(BFD),
        "gamma": ins["gamma"],
        "beta": ins["beta"],
    }
    if Y1_MODE == "half":
        shared["Wt1b"] = np.ascontiguousarray(
            wt1f.reshape(M, EC, P, E)[:, 2:4].astype(BFD)
        )
    return shared


def kernel(**inputs) -> np.ndarray:
    ins = {
        k: np.ascontiguousarray(np.asarray(v, dtype=np.float32))
        for k, v in inputs.items()
    }
    x = ins["x"]
    assert x.shape == (M, B_FULL, E), x.shape
    fast_gb = bool(np.all(ins["gamma"] == 1.0) and np.all(ins["beta"] == 0.0))
    nc = _get_program(BC, fast_gb)

    shared = _prep_shared(ins)
    in_maps = []
    for i in range(N_CORES):
        m = dict(shared)
        m["x"] = np.ascontiguousarray(x[:, i * BC : (i + 1) * BC, :])
        in_maps.append(m)

    res = run_bass_kernel_spmd(nc, in_maps, core_ids=list(range(N_CORES)))
    out = np.concatenate(
        [np.asarray(res.results[i]["out"]) for i in range(N_CORES)], axis=0
    )
    return out.astype(np.float32)


if __name__ == "__main__":
    rng = np.random.default_rng(0)
    ins = {
        "x": rng.standard_normal((M, B_FULL, E), dtype=np.float32),
        "Wq": (rng.standard_normal((M, E, A)) / np.sqrt(E)).astype(np.float32),
        "bq": (rng.standard_normal((M, A)) / np.sqrt(E)).astype(np.float32),
        "Wk": (rng.standard_normal((M, E, A)) / np.sqrt(E)).astype(np.float32),
        "bk": (rng.standard_normal((M, A)) / np.sqrt(E)).astype(np.float32),
        "v": (rng.standard_normal((M, A)) / np.sqrt(A)).astype(np.float32),
        "Wt": (rng.standard_normal((M, M, E, E)) / np.sqrt(E)).astype(np.float32),
        "bt": (rng.standard_normal((M, M, E)) / np.sqrt(E)).astype(np.float32),
        "gamma": np.ones((M * E,), np.float32),
        "beta": np.zeros((M * E,), np.float32),
    }
    out = kernel(**ins)
    print("out", out.shape, out.dtype)
